# revision 1
# baseline (speedup 1.0000x reference)
"""Trainium2 Bass kernel for nn_MultiHeadAttn (conv-QKV multi-head attention).

Sharding: pure data parallelism over batch B=8 -> one batch item per NeuronCore.
Per-core pipeline (all matmuls in float32r = full-rate fp32 mode):
  - 3x3 SAME convs for Q, K, V as 72-step PSUM-accumulated matmuls
    (contraction over (in_channel_chunk, tap)).
    Q/K computed transposed ([pixel, channel] = [feature, token]) directly;
    V computed natural ([token, feature]) with a ones-column appended.
  - Attention in S^T layout: S^T[tk, tq] = K Q^T per head, mask added as
    (m-1)*1e9 pre-exp, exp on ACT with scale=1/8 (no max subtraction needed:
    |logits| <= ~30), PV matmul gives O'^T[d, tq] with softmax denominators in
    row 64 (from V's ones column); normalization via reciprocal + K=1
    broadcast-matmul.
  - Output linear out = O @ Wo^T + bo, accumulated over feature chunks.
Host-side work is layout-only: transposes / padding / slicing of inputs.
"""

import sys

if "/opt/trn_rl_repo" not in sys.path:
    sys.path.insert(0, "/opt/trn_rl_repo")

import numpy as np

_CACHE = {}

B = 8
C = 1024          # tokens (= conv channels)
F = 1024          # features (= H*W pixels)
NH = 16           # heads
HD = 64           # head dim
PAD = 34          # padded spatial width
PA = PAD * PAD    # 1156


def _build_program(reps=1):
    from contextlib import ExitStack

    import concourse.bass as bass
    import concourse.mybir as mybir
    import concourse.tile as tile
    from concourse import bacc

    FP = mybir.dt.float32
    FR = mybir.dt.float32r
    I32 = mybir.dt.int32
    AL = mybir.AluOpType
    AF = mybir.ActivationFunctionType

    nc = bacc.Bacc(None, target_bir_lowering=False)

    # Per-core inputs (host-prepped layouts)
    xq_d = nc.dram_tensor("xq", [C, PA], FR, kind="ExternalInput")   # padded [i, 34*34]
    xk_d = nc.dram_tensor("xk", [C, PA], FR, kind="ExternalInput")
    xv_d = nc.dram_tensor("xv", [C, PA], FR, kind="ExternalInput")
    wq_d = nc.dram_tensor("wq", [8, 9, 128, C], FR, kind="ExternalInput")  # [ic, s, i, o]
    wk_d = nc.dram_tensor("wk", [8, 9, 128, C], FR, kind="ExternalInput")
    wv_d = nc.dram_tensor("wv", [8, 9, 128, C], FR, kind="ExternalInput")
    wo_d = nc.dram_tensor("wo", [F, C], FR, kind="ExternalInput")          # Wo^T [f, j]
    bq_d = nc.dram_tensor("bq", [C], FP, kind="ExternalInput")
    bk_d = nc.dram_tensor("bk", [C], FP, kind="ExternalInput")
    bv_d = nc.dram_tensor("bv", [C], FP, kind="ExternalInput")
    bo_d = nc.dram_tensor("bo", [C], FP, kind="ExternalInput")
    mt_d = nc.dram_tensor("mt", [C, C], I32, kind="ExternalInput")         # mask^T [s, t]
    out_d = nc.dram_tensor("out", [C, C], FP, kind="ExternalOutput")

    with ExitStack() as ctx:
        tc = ctx.enter_context(tile.TileContext(nc))
        for _rep in range(reps):
            _build_body(nc, tc, bass, mybir, tile,
                        (xq_d, xk_d, xv_d, wq_d, wk_d, wv_d, wo_d,
                         bq_d, bk_d, bv_d, bo_d, mt_d, out_d))

    nc.compile()
    return nc


def _build_body(nc, tc, bass, mybir, tile, drams):
    from contextlib import ExitStack

    FP = mybir.dt.float32
    FR = mybir.dt.float32r
    I32 = mybir.dt.int32
    AL = mybir.AluOpType
    AF = mybir.ActivationFunctionType
    (xq_d, xk_d, xv_d, wq_d, wk_d, wv_d, wo_d,
     bq_d, bk_d, bv_d, bo_d, mt_d, out_d) = drams

    def bcast(dram_h):
        ap = dram_h[:]
        return bass.AP(tensor=ap.tensor, offset=ap.offset, ap=[[0, 128]] + list(ap.ap))

    with ExitStack() as ctx:
        persist = ctx.enter_context(tc.tile_pool(name="persist", bufs=1))
        qt = persist.tile([128, 8, C], FR)        # Q^T: [f%128, f//128, t]
        kt = persist.tile([128, 8, C], FR)        # K^T
        vt = [persist.tile([128, NH, HD + 1], FR, name=f"vt{i}")
              for i in range(8)]  # V:[t%128][t//128][h, d] + ones col
        ones_g = persist.tile([128, 128], FP)
        nc.gpsimd.memset(ones_g, 1.0)
        from concourse.masks import make_identity
        ident = persist.tile([128, 128], FP)
        make_identity(nc, ident)
        ident_r = persist.tile([128, 128], FR)
        nc.vector.tensor_copy(out=ident_r, in_=ident)

        amt = persist.tile([128, 8, C], FR)  # (mask^T - 1) * 1e9

        def stage_mask_and_ones():
            for i in range(8):
                nc.vector.tensor_copy(
                    out=vt[i][:, :, HD:HD + 1],
                    in_=ones_g[:, 0:16].rearrange("p (a b) -> p a b", b=1))
            with tc.tile_pool(name="mstp", bufs=2) as mstp:
                for sc in range(8):
                    mst = mstp.tile([128, C], I32, tag="m", name="mst")
                    nc.gpsimd.dma_start(
                        out=mst, in_=mt_d[sc * 128:(sc + 1) * 128, :])
                    nc.vector.tensor_scalar(
                        out=amt[:, sc], in0=mst,
                        scalar1=1e9, scalar2=-1e9, op0=AL.mult, op1=AL.add)

        # ---------------- conv phase ----------------
        with tc.tile_pool(name="convp", bufs=1) as convp, \
                tc.tile_pool(name="stgc", bufs=2) as stgc, \
                tc.tile_pool(name="wpool", bufs=2) as wpool:
            x_pad = convp.tile([128, 8, PA], FR)
            bqp = convp.tile([128, 8], FP)
            bkp = convp.tile([128, 8], FP)
            bvp = convp.tile([128, 8], FP)
            nc.gpsimd.dma_start(out=bqp, in_=bq_d[:].rearrange("(a p) -> p a", p=128))
            nc.gpsimd.dma_start(out=bkp, in_=bk_d[:].rearrange("(a p) -> p a", p=128))
            nc.gpsimd.dma_start(out=bvp, in_=bv_d[:].rearrange("(a p) -> p a", p=128))

            def load_x(xd):
                # x loads on the ACT HWDGE ring so they overlap W loads (sync ring)
                for icc in range(8):
                    nc.scalar.dma_start(
                        out=x_pad[:, icc],
                        in_=xd[icc * 128:(icc + 1) * 128, :],
                    )

            def xwin(icc, dy, dx, y0, ny):
                # shifted conv window: [128, ny, 32] view of padded x
                return x_pad[:, icc].rearrange("p (a b) -> p a b", b=PAD)[
                    :, y0 + dy:y0 + dy + ny, dx:dx + 32]

            def conv_nat(xd, wd, drain):
                """Natural-output conv: psum [o_chunk=t, px=f], drained per t-chunk."""
                load_x(xd)
                for oq in range(4):
                    ps = [psv.tile([128, C], FP, tag="v", name=f"psv{i}") for i in range(2)]
                    for icc in range(8):
                        wt = wpool.tile([128, 9, 256], FR, tag="w")
                        nc.sync.dma_start(
                            out=wt,
                            in_=wd[icc].transpose([1, 0, 2])[:, :, oq * 256:(oq + 1) * 256])
                        for s in range(9):
                            dy, dx = s // 3, s % 3
                            for oc2 in range(2):
                                lhsT = wt[:, s, oc2 * 128:(oc2 + 1) * 128].bitcast(FR)
                                for ph in range(2):
                                    nc.tensor.matmul(
                                        ps[oc2][:, ph * 512:(ph + 1) * 512],
                                        lhsT,
                                        xwin(icc, dy, dx, ph * 16, 16).bitcast(FR),
                                        start=(icc == 0 and s == 0),
                                        stop=(icc == 7 and s == 8))
                    for oc2 in range(2):
                        drain(oq * 2 + oc2, ps[oc2])

            def drain_v(tcc, ps):
                nc.vector.tensor_scalar(
                    out=vt[tcc][:, :, 0:HD],
                    in0=ps.rearrange("p (a b) -> p a b", b=HD),
                    scalar1=bvp[:, tcc:tcc + 1],
                    scalar2=None,
                    op0=AL.add)

            def make_drain_qk(bpp, dst):
                def drain(tcc, ps):
                    stg_t = stgc.tile([128, C], FR, tag="st", name="stg_t")
                    nc.vector.tensor_scalar(
                        out=stg_t, in0=ps,
                        scalar1=bpp[:, tcc:tcc + 1], scalar2=None, op0=AL.add)
                    for fcc in range(8):
                        pt_ps = psT.tile([128, 128], FR, tag="t", name="pt_ps")
                        nc.tensor.transpose(
                            pt_ps, stg_t[:, fcc * 128:(fcc + 1) * 128], ident_r)
                        nc.scalar.copy(
                            out=dst[:, fcc, tcc * 128:(tcc + 1) * 128], in_=pt_ps)
                return drain

            with tc.tile_pool(name="psv", bufs=3, space="PSUM") as psv, \
                    tc.tile_pool(name="psT", bufs=2, space="PSUM") as psT:
                conv_nat(xq_d, wq_d, make_drain_qk(bqp, qt))
                stage_mask_and_ones()
                conv_nat(xk_d, wk_d, make_drain_qk(bkp, kt))
                conv_nat(xv_d, wv_d, drain_v)

        # ---------------- attention + output linear ----------------
        with tc.tile_pool(name="otp", bufs=1) as otp:
            ot = otp.tile([128, 8, C], FR)        # O^T: [f%128, f//128, t]

            with tc.tile_pool(name="attp", bufs=1) as attp, \
                    tc.tile_pool(name="ptp", bufs=6) as ptp, \
                    tc.tile_pool(name="smallp", bufs=2) as smallp, \
                    tc.tile_pool(name="dscp", bufs=4, space="DRAM") as dscp, \
                    tc.tile_pool(name="psS", bufs=4, space="PSUM") as psS, \
                    tc.tile_pool(name="psO", bufs=2, space="PSUM") as psO:
                for fc in range(8):
                    po = {}
                    for hh, pb in ((2 * fc, 0), (2 * fc + 1, 64)):
                        po[hh] = psO.tile([65, C], FP, tag="o", name=f"po{hh}")
                    for tkc in range(8):
                        for hh, pb in ((2 * fc, 0), (2 * fc + 1, 64)):
                            for qh in range(2):
                                s_ps = psS.tile([128, 512], FP, tag="s", name="sps")
                                nc.tensor.matmul(
                                    s_ps,
                                    kt[pb:pb + 64, fc, tkc * 128:(tkc + 1) * 128].bitcast(FR),
                                    qt[pb:pb + 64, fc, qh * 512:(qh + 1) * 512].bitcast(FR),
                                    start=True, stop=(qh == 1))
                                ptt = ptp.tile([128, 512], FR, tag="pt", name="ptt")
                                if qh == 0:
                                    nc.tensor.matmul(
                                        s_ps,
                                        ident_r,
                                        amt[:, tkc, qh * 512:(qh + 1) * 512],
                                        start=False, stop=True)
                                    nc.scalar.activation(
                                        out=ptt, in_=s_ps, func=AF.Exp, scale=0.125)
                                else:
                                    nc.vector.tensor_add(
                                        ptt, s_ps, amt[:, tkc, qh * 512:(qh + 1) * 512])
                                    nc.scalar.activation(
                                        out=ptt, in_=ptt, func=AF.Exp, scale=0.125)
                                nc.tensor.matmul(
                                    po[hh][:, qh * 512:(qh + 1) * 512],
                                    vt[tkc][:, hh].bitcast(FR),
                                    ptt.bitcast(FR),
                                    start=(tkc == 0), stop=(tkc == 7))
                    for hh, pb in ((2 * fc, 0), (2 * fc + 1, 64)):
                        ou = smallp.tile([65, C], FP, tag="ou", name="ou")
                        nc.scalar.copy(out=ou, in_=po[hh])
                        nc.vector.reciprocal(
                            out=ou[64:65, :], in_=ou[64:65, :])
                        dsc = dscp.tile([1, C], FP, tag="d", name="dsc")
                        nc.gpsimd.dma_start(out=dsc, in_=ou[64:65, :])
                        rbs = smallp.tile([64, C], FP, tag="rbs", name="rbs")
                        dap = dsc[0:1, :]
                        nc.gpsimd.dma_start(out=rbs, in_=bass.AP(
                            tensor=dap.tensor, offset=dap.offset,
                            ap=[[0, 64]] + list(dap.ap)[1:]))
                        if pb == 0:
                            nc.vector.tensor_mul(
                                ot[0:64, fc, :], ou[0:64, :], rbs)
                        else:
                            stage = smallp.tile([64, C], FR, tag="sg", name="sg")
                            nc.vector.tensor_mul(
                                stage, ou[0:64, :], rbs)
                            nc.gpsimd.dma_start(
                                out=ot[64:128, fc, :], in_=stage)

            with tc.tile_pool(name="linp", bufs=1) as linp, \
                    tc.tile_pool(name="stg", bufs=2) as stg, \
                    tc.tile_pool(name="psL", bufs=4, space="PSUM") as psL:
                wos = [linp.tile([128, C], FR, name=f"wos{i}") for i in range(8)]
                for fc in range(8):
                    nc.sync.dma_start(
                        out=wos[fc],
                        in_=wo_d[fc * 128:(fc + 1) * 128, :])
                bob = linp.tile([128, C], FP)
                nc.gpsimd.dma_start(out=bob, in_=bcast(bo_d))
                for tg in range(2):
                    pls = [psL.tile([128, C], FP, tag="l", name=f"psl{i}") for i in range(4)]
                    for fc in range(8):
                        for t4 in range(4):
                            tcc = tg * 4 + t4
                            lhsT = ot[:, fc, tcc * 128:(tcc + 1) * 128].bitcast(FR)
                            for jh in range(2):
                                nc.tensor.matmul(
                                    pls[t4][:, jh * 512:(jh + 1) * 512],
                                    lhsT,
                                    wos[fc][:, jh * 512:(jh + 1) * 512],
                                    start=(fc == 0), stop=(fc == 7))
                    for t4 in range(4):
                        tcc = tg * 4 + t4
                        so = stg.tile([128, C], FP, tag="so", name="so")
                        nc.vector.tensor_add(so, pls[t4], bob)
                        nc.sync.dma_start(
                            out=out_d[tcc * 128:(tcc + 1) * 128, :], in_=so)


def _prep_w(W):
    # [O, I, 3, 3] -> [ic, s, i_local, o] contiguous
    Wt = W.transpose(1, 2, 3, 0).reshape(8, 128, 9, C)
    return np.ascontiguousarray(Wt.transpose(0, 2, 1, 3))


def _prep_x(x):
    # [C, 32, 32] -> zero-padded [C, 34*34]
    xp = np.zeros((C, PAD, PAD), np.float32)
    xp[:, 1:33, 1:33] = x
    return xp.reshape(C, PA)


def get_program(reps=1):
    key = ("nc", reps)
    if key not in _CACHE:
        _CACHE[key] = _build_program(reps)
    return _CACHE[key]


def make_in_maps(q, k, v, Wq, bq, Wk, bk, Wv, bv, Wo, bo, mask):
    wq = _prep_w(np.asarray(Wq))
    wk = _prep_w(np.asarray(Wk))
    wv = _prep_w(np.asarray(Wv))
    wo = np.ascontiguousarray(np.asarray(Wo).T)
    bq, bk, bv, bo = (np.ascontiguousarray(np.asarray(b), dtype=np.float32)
                      for b in (bq, bk, bv, bo))
    in_maps = []
    for b in range(B):
        in_maps.append({
            "xq": _prep_x(np.asarray(q[b]).reshape(C, 32, 32)),
            "xk": _prep_x(np.asarray(k[b]).reshape(C, 32, 32)),
            "xv": _prep_x(np.asarray(v[b]).reshape(C, 32, 32)),
            "wq": wq, "wk": wk, "wv": wv, "wo": wo,
            "bq": bq, "bk": bk, "bv": bv, "bo": bo,
            "mt": np.ascontiguousarray(np.asarray(mask[b]).T),
        })
    return in_maps


def run(inputs, trace=False, **kw):
    from concourse.bass_utils import run_bass_kernel_spmd

    nc = get_program()
    in_maps = make_in_maps(**inputs)
    res = run_bass_kernel_spmd(nc, in_maps, list(range(B)), trace=trace, **kw)
    out = np.stack([res.results[i]["out"] for i in range(B)], axis=0)
    return out, res


def kernel(**inputs) -> np.ndarray:
    out, _ = run(inputs, trace=False)
    return out



# revision 15
# speedup vs baseline: 1.3139x; 1.3139x over previous
"""Trainium2 Bass kernel for nn_MultiHeadAttn (conv-QKV multi-head attention).

Sharding: data parallel over batch B=8 -> one batch item per NeuronCore.

Per-core pipeline:
  - 3x3 SAME convs for Q/K/V as fp8(e4m3) DoubleRow matmuls: host splits
    x*16 and W*512 into (hi, lo) e4m3 pairs; the 3-product scheme
    (W0x0 + W0x1 + W1x0) runs at 1.5 DoubleRow instructions per 128-deep
    k-chunk = 0.75 cycles/output-column (vs 1.0 for fp32r), pair packing
    done purely by AP strides:
      DR_a(icc) = lhsT (W0,W1) x rhs (x0, x0dup)  -> W0x0 + W1x0
      DR_c(iccA,iccB) = lhsT (W0A,W0B) x rhs (x1A,x1B) -> cross terms
    Conv windows are flat contiguous runs over the 34x34 zero-padded image
    (pad columns produce garbage output columns that drains skip).
  - Q/K drains: descale+bias on DVE -> fp16 staging -> XBAR DMA-transpose
    into [f, t] layout (no PE transposes).
  - Attention: S^T = K Q^T per head in fp16 (64-deep); mask added on the
    Pool engine from a bf16 (mask^T-1)*1e9 tile; exp on ACT batched over
    tkc pairs -> bf16 P^T.  PV in swapped orientation: out[tq, 65] =
    (P^T-chunk)^T @ V with a ones column giving the softmax denominator;
    4 accumulation groups share one PSUM bank (memset + start=False).
    Normalize via reciprocal + per-partition scale, DMA-transpose to O^T.
  - Conv order V -> K -> Q; attention qh=0 is interleaved with Q-conv
    output chunks 4..7 so its Pool/ACT chain hides under conv PE work;
    qh=1 is interleaved with the first half of the output linear.
  - Output linear in bf16: out = O @ Wo^T + bo.
"""

import sys

if "/opt/trn_rl_repo" not in sys.path:
    sys.path.insert(0, "/opt/trn_rl_repo")

import numpy as np

_CACHE = {}

B = 8
C = 1024          # tokens (= conv channels)
F = 1024          # features (= H*W pixels)
NH = 16           # heads
HD = 64           # head dim
PADW = 33         # padded row width (left pad doubles as prev row's right pad)
PADH = 35         # padded rows incl top + 2 bottom (tap-overrun tail)
PA = PADH * PADW  # 1155
SX = 16.0         # fp8 scale for x
SW = 512.0        # fp8 scale for W
DESCALE = 1.0 / (SX * SW)
BLOCKS = [(0, 512), (512, 512), (1024, 31)]   # flat-window column blocks (1055)


def _build_program():
    from contextlib import ExitStack

    import concourse.bass as bass
    import concourse.mybir as mybir
    import concourse.tile as tile
    from concourse import bacc

    FP = mybir.dt.float32
    F8 = mybir.dt.float8e4
    BF = mybir.dt.bfloat16

    nc = bacc.Bacc(None, target_bir_lowering=False)

    xq_d = nc.dram_tensor("xq", [128, 8, 3, PA], F8, kind="ExternalInput")
    xk_d = nc.dram_tensor("xk", [128, 8, 3, PA], F8, kind="ExternalInput")
    xv_d = nc.dram_tensor("xv", [128, 8, 3, PA], F8, kind="ExternalInput")
    wq_d = nc.dram_tensor("wq", [32, 128, 4608], F8, kind="ExternalInput")
    wk_d = nc.dram_tensor("wk", [32, 128, 4608], F8, kind="ExternalInput")
    wv_d = nc.dram_tensor("wv", [32, 128, 4608], F8, kind="ExternalInput")
    wo_d = nc.dram_tensor("wo", [F, C], BF, kind="ExternalInput")        # Wo^T
    bq_d = nc.dram_tensor("bq", [C], FP, kind="ExternalInput")
    bk_d = nc.dram_tensor("bk", [C], FP, kind="ExternalInput")
    bv_d = nc.dram_tensor("bv", [C], FP, kind="ExternalInput")
    bo_d = nc.dram_tensor("bo", [C], FP, kind="ExternalInput")
    mt_d = nc.dram_tensor("mt", [C, C], BF, kind="ExternalInput")        # m^T in {0,1}
    out_d = nc.dram_tensor("out", [C, C], FP, kind="ExternalOutput")

    with ExitStack() as ctx:
        tc = ctx.enter_context(tile.TileContext(nc))
        _build_body(nc, tc, bass, mybir, tile,
                    (xq_d, xk_d, xv_d, wq_d, wk_d, wv_d, wo_d,
                     bq_d, bk_d, bv_d, bo_d, mt_d, out_d))

    nc.compile()
    return nc


def _build_body(nc, tc, bass, mybir, tile, drams):
    from contextlib import ExitStack

    FP = mybir.dt.float32
    F8 = mybir.dt.float8e4
    F16 = mybir.dt.float16
    BF = mybir.dt.bfloat16
    AL = mybir.AluOpType
    AF = mybir.ActivationFunctionType
    DR = mybir.MatmulPerfMode.DoubleRow
    (xq_d, xk_d, xv_d, wq_d, wk_d, wv_d, wo_d,
     bq_d, bk_d, bv_d, bo_d, mt_d, out_d) = drams

    def bcast(dram_h):
        ap = dram_h[:]
        return bass.AP(tensor=ap.tensor, offset=ap.offset,
                       ap=[[0, 128]] + list(ap.ap))

    def sub_ap(t_ap, extra_off, dims):
        return bass.AP(tensor=t_ap.tensor, offset=t_ap.offset + extra_off,
                       ap=[list(t_ap.ap[0])] + dims)

    with ExitStack() as ctx:
        persist = ctx.enter_context(tc.tile_pool(name="persist", bufs=1))
        qt = persist.tile([128, 8, C], F16)       # Q^T: [f%128, f//128, t]
        kt = persist.tile([128, 8, C], F16)       # K^T
        ot = persist.tile([128, 8, C], BF)        # O^T: [f%128, f//128, t]
        vt = [persist.tile([128, NH, HD + 1], BF, name=f"vt{i}")
              for i in range(8)]                  # V natural + ones col
        amt = persist.tile([128, 8, C], BF)       # m^T in {0,1}

        onesbf = persist.tile([128, 16], BF)
        nc.gpsimd.memset(onesbf, 1.0)
        nc.gpsimd.dma_start(out=amt, in_=mt_d[:].rearrange(
            "(a p) t -> p a t", p=128))
        for i in range(8):
            nc.vector.tensor_copy(
                out=vt[i][:, :, HD:HD + 1],
                in_=onesbf.rearrange("p (a b) -> p a b", b=1))

        convp = ctx.enter_context(tc.tile_pool(name="convp", bufs=1))
        xpool = ctx.enter_context(tc.tile_pool(name="xpool", bufs=2))
        stgc = ctx.enter_context(tc.tile_pool(name="stgc", bufs=2))
        wpool = ctx.enter_context(tc.tile_pool(name="wpool", bufs=3))

        bqp = convp.tile([128, 8], FP)
        bkp = convp.tile([128, 8], FP)
        bvp = convp.tile([128, 8], FP)
        nc.gpsimd.dma_start(out=bqp, in_=bq_d[:].rearrange("(a p) -> p a", p=128))
        nc.gpsimd.dma_start(out=bkp, in_=bk_d[:].rearrange("(a p) -> p a", p=128))
        nc.gpsimd.dma_start(out=bvp, in_=bv_d[:].rearrange("(a p) -> p a", p=128))

        def load_x(xd):
            xt = xpool.tile([128, 8, 3, PA], F8, tag="x", name="xt")
            for j in range(4):
                nc.scalar.dma_start(out=xt[:, 2 * j:2 * j + 2],
                                    in_=xd[:, 2 * j:2 * j + 2])
            return xt

        def conv_chunk(xt, wd, oc4, drain, psv):
            """One 128-out-channel chunk of a conv (fp8 DoubleRow)."""
            xap = xt[:]
            ps = psv.tile([128, 1536], FP, tag="v", name="psv")
            for icc2 in range(4):
                wt = wpool.tile([128, 2, 2, 9, 128], F8, tag="w", name="wt")
                nc.sync.dma_start(
                    out=wt[:].rearrange("p a b c d -> p (a b c d)"),
                    in_=wd[oc4 * 4 + icc2])
                for s in range(9):
                    off = (s // 3) * PADW + (s % 3)
                    first = (icc2 == 0 and s == 0)
                    last = (icc2 == 3 and s == 8)
                    for bo_, bw in BLOCKS:
                        for i in range(2):
                            icc = icc2 * 2 + i
                            nc.tensor.matmul(
                                ps[:, bo_:bo_ + bw],
                                wt[:, i, :, s, :],
                                sub_ap(xap, icc * 3 * PA + off + bo_,
                                       [[2 * PA, 2], [1, bw]]),
                                start=(first and i == 0), stop=False,
                                perf_mode=DR)
                        nc.tensor.matmul(
                            ps[:, bo_:bo_ + bw],
                            wt[:, :, 0, s, :],
                            sub_ap(xap, icc2 * 6 * PA + PA + off + bo_,
                                   [[3 * PA, 2], [1, bw]]),
                            start=False, stop=last,
                            perf_mode=DR)
            drain(oc4, ps)

        def make_drain_qk(bpp, dst):
            def drain(oc4, ps):
                stg = stgc.tile([128, 1024], F16, tag="st", name="stg")
                psap = ps[:]
                nc.vector.tensor_scalar(
                    out=stg.rearrange("p (a b) -> p a b", b=32),
                    in0=sub_ap(psap, 0, [[33, 32], [1, 32]]),
                    scalar1=DESCALE, scalar2=bpp[:, oc4:oc4 + 1],
                    op0=AL.mult, op1=AL.add)
                # XBAR transpose: stg [t, f] -> dst[f%128, f//128, t-chunk]
                dstap = dst[:]
                nc.sync.dma_start_transpose(
                    sub_ap(dstap, oc4 * 128, [[C, 8], [1, 128]]), stg)
            return drain

        def drain_v(oc4, ps):
            psap = ps[:]
            nc.vector.tensor_scalar(
                out=vt[oc4][:, :, 0:HD],
                in0=sub_ap(psap, 0, [[66, 16], [33, 2], [1, 32]]),
                scalar1=DESCALE, scalar2=bvp[:, oc4:oc4 + 1],
                op0=AL.mult, op1=AL.add)

        drain_q = make_drain_qk(bqp, qt)
        drain_k = make_drain_qk(bkp, kt)

        # ---------------- phase A: V, K convs + Q chunks 0-3 ----------------
        xtv = load_x(xv_d)
        xtk = load_x(xk_d)
        with tc.tile_pool(name="psvA", bufs=2, space="PSUM") as psvA:
            for oc4 in range(8):
                conv_chunk(xtv, wv_d, oc4, drain_v, psvA)
            xtq = load_x(xq_d)
            for oc4 in range(8):
                conv_chunk(xtk, wk_d, oc4, drain_k, psvA)
            for oc4 in range(4):
                conv_chunk(xtq, wq_d, oc4, drain_q, psvA)

        # ---------------- phase B: Q chunks 4-7 + attention + linear --------
        with tc.tile_pool(name="ptp", bufs=2) as ptp, \
                tc.tile_pool(name="smallp", bufs=3) as smallp, \
                tc.tile_pool(name="psO", bufs=1, space="PSUM") as psO:

            def attn_pair(qh, fc, psSp, batched):
                """Both heads of f-chunk fc: S+exp+mask+PV per head, then a
                combined drain through one [128,512] XBAR transpose."""
                pos = []
                for sub in range(2):
                    hh = 2 * fc + sub
                    pb = sub * 64
                    po = psO.tile([128, 512], FP, tag=f"po{sub}",
                                  name=f"po{sub}")
                    pos.append(po)
                    nc.vector.memset(po[:, 0:4 * (HD + 1)], 0.0)
                    for tk2 in range(4):
                        if batched:
                            s_ps = psSp.tile([128, 1024], FP, tag="s",
                                             name="sps")
                            for h2 in range(2):
                                tkc = tk2 * 2 + h2
                                nc.tensor.matmul(
                                    s_ps[:, h2 * 512:(h2 + 1) * 512],
                                    kt[pb:pb + 64, fc,
                                       tkc * 128:(tkc + 1) * 128],
                                    qt[pb:pb + 64, fc,
                                       qh * 512:(qh + 1) * 512],
                                    start=True, stop=True)
                            praw = ptp.tile([128, 1024], BF, tag="pr",
                                            name="praw")
                            nc.scalar.activation(
                                out=praw, in_=s_ps, func=AF.Exp, scale=0.125)
                        else:
                            praw = ptp.tile([128, 1024], BF, tag="pr",
                                            name="praw")
                            for h2 in range(2):
                                tkc = tk2 * 2 + h2
                                s_ps = psSp.tile([128, 512], FP, tag="s",
                                                 name="sps")
                                nc.tensor.matmul(
                                    s_ps,
                                    kt[pb:pb + 64, fc,
                                       tkc * 128:(tkc + 1) * 128],
                                    qt[pb:pb + 64, fc,
                                       qh * 512:(qh + 1) * 512],
                                    start=True, stop=True)
                                nc.scalar.activation(
                                    out=praw[:, h2 * 512:(h2 + 1) * 512],
                                    in_=s_ps, func=AF.Exp, scale=0.125)
                        ptt = ptp.tile([128, 1024], BF, tag="pt", name="ptt")
                        nc.gpsimd.tensor_tensor(
                            out=ptt.rearrange("p (a b) -> p a b", b=512),
                            in0=praw.rearrange("p (a b) -> p a b", b=512),
                            in1=amt[:, tk2 * 2:tk2 * 2 + 2,
                                    qh * 512:(qh + 1) * 512],
                            op=AL.mult)
                        for h2 in range(2):
                            tkc = tk2 * 2 + h2
                            for t4 in range(4):
                                nc.tensor.matmul(
                                    po[:, t4 * (HD + 1):(t4 + 1) * (HD + 1)],
                                    ptt[:, h2 * 512 + t4 * 128:
                                        h2 * 512 + (t4 + 1) * 128],
                                    vt[tkc][:, hh, :],
                                    start=False, stop=(tkc == 7),
                                    skip_group_check=True)
                # combined drain
                rd = smallp.tile([128, 8], FP, tag="rd", name="rd")
                onorm = smallp.tile([128, 512], BF, tag="on", name="onorm")
                for sub in range(2):
                    poap = pos[sub][:]
                    nc.vector.reciprocal(
                        out=rd[:, 4 * sub:4 * sub + 4],
                        in_=sub_ap(poap, HD, [[HD + 1, 4]]))
                    for t4 in range(4):
                        nc.vector.tensor_scalar(
                            out=onorm[:, t4 * 128 + sub * HD:
                                      t4 * 128 + (sub + 1) * HD],
                            in0=pos[sub][:, t4 * (HD + 1):t4 * (HD + 1) + HD],
                            scalar1=rd[:, 4 * sub + t4:4 * sub + t4 + 1],
                            scalar2=None, op0=AL.mult)
                nc.sync.dma_start_transpose(
                    ot[:, fc, qh * 512:(qh + 1) * 512].rearrange(
                        "p (a b) -> p a b", b=128),
                    onorm)

            # NOTE: interleaving conv chunks with attention pairs on the PE
            # stream crashes real hardware (sim-clean); keep them sequential.
            with tc.tile_pool(name="psvB", bufs=1, space="PSUM") as psvB, \
                    tc.tile_pool(name="psS", bufs=2, space="PSUM") as psS:
                for oc4 in range(4, 8):
                    conv_chunk(xtq, wq_d, oc4, drain_q, psvB)
                for fc in range(8):
                    attn_pair(0, fc, psS, False)

            # qh=1 heads interleaved with first half of the output linear
            with tc.tile_pool(name="linp", bufs=1) as linp, \
                    tc.tile_pool(name="stg2", bufs=2) as stg2, \
                    tc.tile_pool(name="psL", bufs=1, space="PSUM") as psL:
                wos = [linp.tile([128, C], BF, name=f"wos{i}") for i in range(8)]
                for fcc in range(8):
                    nc.sync.dma_start(
                        out=wos[fcc], in_=wo_d[fcc * 128:(fcc + 1) * 128, :])
                bob = linp.tile([128, C], FP)
                nc.gpsimd.dma_start(out=bob, in_=bcast(bo_d))

                def linear_tcc(tcc, pool):
                    pl = pool.tile([128, C], FP, tag="l", name="pl")
                    for fcc in range(8):
                        lhsT = ot[:, fcc, tcc * 128:(tcc + 1) * 128]
                        for jh in range(2):
                            nc.tensor.matmul(
                                pl[:, jh * 512:(jh + 1) * 512],
                                lhsT,
                                wos[fcc][:, jh * 512:(jh + 1) * 512],
                                start=(fcc == 0), stop=(fcc == 7))
                    so = stg2.tile([128, C], FP, tag="so", name="so")
                    nc.vector.tensor_add(so, pl, bob)
                    nc.scalar.dma_start(
                        out=out_d[tcc * 128:(tcc + 1) * 128, :], in_=so)

                with tc.tile_pool(name="psS2", bufs=2,
                                  space="PSUM") as psS2:
                    for fc in range(8):
                        attn_pair(1, fc, psS2, True)
                        if fc % 2 == 1:
                            linear_tcc(fc // 2, psL)
                with tc.tile_pool(name="psL2", bufs=2, space="PSUM") as psL2:
                    for tcc in range(4, 8):
                        linear_tcc(tcc, psL2)


def _prep_w(W):
    """[O, I, 3, 3] fp32 -> [32, 128, 4608] fp8 tiles.

    Tile t = oc4*4+icc2, per-partition content [icc(2), hilo(2), s(9), oc(128)].
    """
    import ml_dtypes
    E4 = ml_dtypes.float8_e4m3
    Ws = np.asarray(W, np.float32) * SW
    W0 = Ws.astype(E4)
    W1 = (Ws - W0.astype(np.float32)).astype(E4)
    # [hilo, O, I, 3*3] -> [hilo, I, s, O]
    pair = np.stack([W0, W1]).reshape(2, 1024, 1024, 9).transpose(0, 2, 3, 1)
    # I = icc2*256 + icc*128 + p ; O = oc4*128 + oc
    pair = pair.reshape(2, 4, 2, 128, 9, 8, 128)     # [h, icc2, icc, p, s, oc4, oc]
    out = pair.transpose(5, 1, 3, 2, 0, 4, 6)        # [oc4, icc2, p, icc, h, s, oc]
    return np.ascontiguousarray(out.reshape(32, 128, 4608))


def _prep_x(x):
    """[C, 32, 32] fp32 -> [128, 3, 8, PA] fp8 (slots x0, x1, x0)."""
    import ml_dtypes
    E4 = ml_dtypes.float8_e4m3
    xp = np.zeros((C, PADH, PADW), np.float32)
    xp[:, 1:33, 1:33] = np.asarray(x, np.float32)
    xs = (xp * SX).reshape(C, PA)
    x0 = xs.astype(E4)
    x1 = (xs - x0.astype(np.float32)).astype(E4)
    tri = np.stack([x0, x1, x0], axis=1)             # [C, 3, PA]
    tri = tri.reshape(8, 128, 3, PA).transpose(1, 0, 2, 3)  # [p, icc, slot, PA]
    return np.ascontiguousarray(tri)


def get_program():
    if "nc" not in _CACHE:
        _CACHE["nc"] = _build_program()
    return _CACHE["nc"]


def make_in_maps(q, k, v, Wq, bq, Wk, bk, Wv, bv, Wo, bo, mask):
    import ml_dtypes
    BF16 = ml_dtypes.bfloat16
    wq = _prep_w(Wq)
    wk = _prep_w(Wk)
    wv = _prep_w(Wv)
    wo = np.ascontiguousarray(np.asarray(Wo, np.float32).T.astype(BF16))
    bq, bk, bv, bo = (np.ascontiguousarray(np.asarray(b), dtype=np.float32)
                      for b in (bq, bk, bv, bo))
    in_maps = []
    for b in range(B):
        mt = np.asarray(mask[b]).T.astype(np.float32).astype(BF16)
        in_maps.append({
            "xq": _prep_x(np.asarray(q[b]).reshape(C, 32, 32)),
            "xk": _prep_x(np.asarray(k[b]).reshape(C, 32, 32)),
            "xv": _prep_x(np.asarray(v[b]).reshape(C, 32, 32)),
            "wq": wq, "wk": wk, "wv": wv, "wo": wo,
            "bq": bq, "bk": bk, "bv": bv, "bo": bo,
            "mt": np.ascontiguousarray(mt),
        })
    return in_maps


def run(inputs, trace=False, **kw):
    from concourse.bass_utils import run_bass_kernel_spmd

    nc = get_program()
    in_maps = make_in_maps(**inputs)
    res = run_bass_kernel_spmd(nc, in_maps, list(range(B)), trace=trace, **kw)
    out = np.stack([res.results[i]["out"] for i in range(B)], axis=0)
    return out, res


def kernel(**inputs) -> np.ndarray:
    out, _ = run(inputs, trace=False)
    return out


# revision 16
# speedup vs baseline: 1.3285x; 1.0111x over previous
"""Trainium2 Bass kernel for nn_MultiHeadAttn (conv-QKV multi-head attention).

Sharding: data parallel over batch B=8 -> one batch item per NeuronCore.

Per-core pipeline:
  - 3x3 SAME convs for Q/K/V as fp8(e4m3) DoubleRow matmuls: host splits
    x*16 and W*512 into (hi, lo) e4m3 pairs; the 3-product scheme
    (W0x0 + W0x1 + W1x0) runs at 1.5 DoubleRow instructions per 128-deep
    k-chunk = 0.75 cycles/output-column (vs 1.0 for fp32r), pair packing
    done purely by AP strides:
      DR_a(icc) = lhsT (W0,W1) x rhs (x0, x0dup)  -> W0x0 + W1x0
      DR_c(iccA,iccB) = lhsT (W0A,W0B) x rhs (x1A,x1B) -> cross terms
    Conv windows are flat contiguous runs over the 34x34 zero-padded image
    (pad columns produce garbage output columns that drains skip).
  - Q/K drains: descale+bias on DVE -> fp16 staging -> XBAR DMA-transpose
    into [f, t] layout (no PE transposes).
  - Attention: S^T = K Q^T per head in fp16 (64-deep); mask added on the
    Pool engine from a bf16 (mask^T-1)*1e9 tile; exp on ACT batched over
    tkc pairs -> bf16 P^T.  PV in swapped orientation: out[tq, 65] =
    (P^T-chunk)^T @ V with a ones column giving the softmax denominator;
    4 accumulation groups share one PSUM bank (memset + start=False).
    Normalize via reciprocal + per-partition scale, DMA-transpose to O^T.
  - Conv order V -> K -> Q; attention qh=0 is interleaved with Q-conv
    output chunks 4..7 so its Pool/ACT chain hides under conv PE work;
    qh=1 is interleaved with the first half of the output linear.
  - Output linear in bf16: out = O @ Wo^T + bo.
"""

import sys

if "/opt/trn_rl_repo" not in sys.path:
    sys.path.insert(0, "/opt/trn_rl_repo")

import numpy as np

_CACHE = {}

B = 8
C = 1024          # tokens (= conv channels)
F = 1024          # features (= H*W pixels)
NH = 16           # heads
HD = 64           # head dim
PADW = 33         # padded row width (left pad doubles as prev row's right pad)
PADH = 35         # padded rows incl top + 2 bottom (tap-overrun tail)
PA = PADH * PADW  # 1155
SX = 16.0         # fp8 scale for x
SW = 512.0        # fp8 scale for W
DESCALE = 1.0 / (SX * SW)
BLOCKS = [(0, 512), (512, 512), (1024, 31)]   # flat-window column blocks (1055)


def _build_program():
    from contextlib import ExitStack

    import concourse.bass as bass
    import concourse.mybir as mybir
    import concourse.tile as tile
    from concourse import bacc

    FP = mybir.dt.float32
    F8 = mybir.dt.float8e4
    BF = mybir.dt.bfloat16

    nc = bacc.Bacc(None, target_bir_lowering=False)

    xq_d = nc.dram_tensor("xq", [128, 8, 3, PA], F8, kind="ExternalInput")
    xk_d = nc.dram_tensor("xk", [128, 8, 3, PA], F8, kind="ExternalInput")
    xv_d = nc.dram_tensor("xv", [128, 8, 3, PA], F8, kind="ExternalInput")
    wq_d = nc.dram_tensor("wq", [32, 128, 4608], F8, kind="ExternalInput")
    wk_d = nc.dram_tensor("wk", [32, 128, 4608], F8, kind="ExternalInput")
    wv_d = nc.dram_tensor("wv", [32, 128, 4608], F8, kind="ExternalInput")
    wo_d = nc.dram_tensor("wo", [F, C], BF, kind="ExternalInput")        # Wo^T
    bq_d = nc.dram_tensor("bq", [C], FP, kind="ExternalInput")
    bk_d = nc.dram_tensor("bk", [C], FP, kind="ExternalInput")
    bv_d = nc.dram_tensor("bv", [C], FP, kind="ExternalInput")
    bo_d = nc.dram_tensor("bo", [C], FP, kind="ExternalInput")
    mt_d = nc.dram_tensor("mt", [C, C], BF, kind="ExternalInput")        # m^T in {0,1}
    out_d = nc.dram_tensor("out", [C, C], FP, kind="ExternalOutput")

    with ExitStack() as ctx:
        tc = ctx.enter_context(tile.TileContext(nc))
        _build_body(nc, tc, bass, mybir, tile,
                    (xq_d, xk_d, xv_d, wq_d, wk_d, wv_d, wo_d,
                     bq_d, bk_d, bv_d, bo_d, mt_d, out_d))

    nc.compile()
    return nc


def _build_body(nc, tc, bass, mybir, tile, drams):
    from contextlib import ExitStack

    FP = mybir.dt.float32
    F8 = mybir.dt.float8e4
    F16 = mybir.dt.float16
    BF = mybir.dt.bfloat16
    AL = mybir.AluOpType
    AF = mybir.ActivationFunctionType
    DR = mybir.MatmulPerfMode.DoubleRow
    (xq_d, xk_d, xv_d, wq_d, wk_d, wv_d, wo_d,
     bq_d, bk_d, bv_d, bo_d, mt_d, out_d) = drams

    def bcast(dram_h):
        ap = dram_h[:]
        return bass.AP(tensor=ap.tensor, offset=ap.offset,
                       ap=[[0, 128]] + list(ap.ap))

    def sub_ap(t_ap, extra_off, dims):
        return bass.AP(tensor=t_ap.tensor, offset=t_ap.offset + extra_off,
                       ap=[list(t_ap.ap[0])] + dims)

    with ExitStack() as ctx:
        persist = ctx.enter_context(tc.tile_pool(name="persist", bufs=1))
        qt = persist.tile([128, 8, C], F16)       # Q^T: [f%128, f//128, t]
        kt = persist.tile([128, 8, C], F16)       # K^T
        ot = persist.tile([128, 8, C], BF)        # O^T: [f%128, f//128, t]
        vt = [persist.tile([128, NH, HD + 1], BF, name=f"vt{i}")
              for i in range(8)]                  # V natural + ones col
        amt = persist.tile([128, 8, C], BF)       # m^T in {0,1}

        onesbf = persist.tile([128, 16], BF)
        nc.gpsimd.memset(onesbf, 1.0)
        nc.gpsimd.dma_start(out=amt, in_=mt_d[:].rearrange(
            "(a p) t -> p a t", p=128))
        for i in range(8):
            nc.vector.tensor_copy(
                out=vt[i][:, :, HD:HD + 1],
                in_=onesbf.rearrange("p (a b) -> p a b", b=1))

        convp = ctx.enter_context(tc.tile_pool(name="convp", bufs=1))
        xpool = ctx.enter_context(tc.tile_pool(name="xpool", bufs=2))
        stgc = ctx.enter_context(tc.tile_pool(name="stgc", bufs=2))
        wpool = ctx.enter_context(tc.tile_pool(name="wpool", bufs=3))

        bqp = convp.tile([128, 8], FP)
        bkp = convp.tile([128, 8], FP)
        bvp = convp.tile([128, 8], FP)
        nc.gpsimd.dma_start(out=bqp, in_=bq_d[:].rearrange("(a p) -> p a", p=128))
        nc.gpsimd.dma_start(out=bkp, in_=bk_d[:].rearrange("(a p) -> p a", p=128))
        nc.gpsimd.dma_start(out=bvp, in_=bv_d[:].rearrange("(a p) -> p a", p=128))

        def load_x(xd):
            xt = xpool.tile([128, 8, 3, PA], F8, tag="x", name="xt")
            for j in range(4):
                nc.scalar.dma_start(out=xt[:, 2 * j:2 * j + 2],
                                    in_=xd[:, 2 * j:2 * j + 2])
            return xt

        def conv_chunk(xt, wd, oc4, drain, psv):
            """One 128-out-channel chunk of a conv (fp8 DoubleRow)."""
            xap = xt[:]
            ps = psv.tile([128, 1536], FP, tag="v", name="psv")
            for icc2 in range(4):
                wt = wpool.tile([128, 2, 2, 9, 128], F8, tag="w", name="wt")
                nc.sync.dma_start(
                    out=wt[:].rearrange("p a b c d -> p (a b c d)"),
                    in_=wd[oc4 * 4 + icc2])
                for s in range(9):
                    off = (s // 3) * PADW + (s % 3)
                    first = (icc2 == 0 and s == 0)
                    last = (icc2 == 3 and s == 8)
                    for bo_, bw in BLOCKS:
                        for i in range(2):
                            icc = icc2 * 2 + i
                            nc.tensor.matmul(
                                ps[:, bo_:bo_ + bw],
                                wt[:, i, :, s, :],
                                sub_ap(xap, icc * 3 * PA + off + bo_,
                                       [[2 * PA, 2], [1, bw]]),
                                start=(first and i == 0), stop=False,
                                perf_mode=DR)
                        nc.tensor.matmul(
                            ps[:, bo_:bo_ + bw],
                            wt[:, :, 0, s, :],
                            sub_ap(xap, icc2 * 6 * PA + PA + off + bo_,
                                   [[3 * PA, 2], [1, bw]]),
                            start=False, stop=last,
                            perf_mode=DR)
            drain(oc4, ps)

        def make_drain_qk(bpp, dst):
            def drain(oc4, ps):
                stg = stgc.tile([128, 1024], F16, tag="st", name="stg")
                psap = ps[:]
                nc.vector.tensor_scalar(
                    out=stg.rearrange("p (a b) -> p a b", b=32),
                    in0=sub_ap(psap, 0, [[33, 32], [1, 32]]),
                    scalar1=DESCALE, scalar2=bpp[:, oc4:oc4 + 1],
                    op0=AL.mult, op1=AL.add)
                # XBAR transpose: stg [t, f] -> dst[f%128, f//128, t-chunk]
                dstap = dst[:]
                nc.sync.dma_start_transpose(
                    sub_ap(dstap, oc4 * 128, [[C, 8], [1, 128]]), stg)
            return drain

        def drain_v(oc4, ps):
            psap = ps[:]
            nc.vector.tensor_scalar(
                out=vt[oc4][:, :, 0:HD],
                in0=sub_ap(psap, 0, [[66, 16], [33, 2], [1, 32]]),
                scalar1=DESCALE, scalar2=bvp[:, oc4:oc4 + 1],
                op0=AL.mult, op1=AL.add)

        drain_q = make_drain_qk(bqp, qt)
        drain_k = make_drain_qk(bkp, kt)

        # ---------------- phase A: V, K convs + Q chunks 0-3 ----------------
        xtv = load_x(xv_d)
        xtk = load_x(xk_d)
        with tc.tile_pool(name="psvA", bufs=2, space="PSUM") as psvA:
            for oc4 in range(8):
                conv_chunk(xtv, wv_d, oc4, drain_v, psvA)
            xtq = load_x(xq_d)
            for oc4 in range(8):
                conv_chunk(xtk, wk_d, oc4, drain_k, psvA)
            for oc4 in range(4):
                conv_chunk(xtq, wq_d, oc4, drain_q, psvA)

        # ---------------- phase B: Q chunks 4-7 + attention + linear --------
        with tc.tile_pool(name="ptp", bufs=2) as ptp, \
                tc.tile_pool(name="smallp", bufs=3) as smallp, \
                tc.tile_pool(name="psO", bufs=1, space="PSUM") as psO:

            def attn_pair(qh, fc, psSp, batched):
                """Both heads of f-chunk fc: S+exp+mask+PV per head, then a
                combined drain through one [128,512] XBAR transpose."""
                pos = []
                for sub in range(2):
                    hh = 2 * fc + sub
                    pb = sub * 64
                    po = psO.tile([128, 512], FP, tag=f"po{sub}",
                                  name=f"po{sub}")
                    pos.append(po)
                    nc.vector.memset(po[:, 0:4 * (HD + 1)], 0.0)
                    for tk2 in range(4):
                        if batched:
                            s_ps = psSp.tile([128, 1024], FP, tag="s",
                                             name="sps")
                            for h2 in range(2):
                                tkc = tk2 * 2 + h2
                                nc.tensor.matmul(
                                    s_ps[:, h2 * 512:(h2 + 1) * 512],
                                    kt[pb:pb + 64, fc,
                                       tkc * 128:(tkc + 1) * 128],
                                    qt[pb:pb + 64, fc,
                                       qh * 512:(qh + 1) * 512],
                                    start=True, stop=True)
                            praw = ptp.tile([128, 1024], BF, tag="pr",
                                            name="praw")
                            nc.scalar.activation(
                                out=praw, in_=s_ps, func=AF.Exp, scale=0.125)
                        else:
                            praw = ptp.tile([128, 1024], BF, tag="pr",
                                            name="praw")
                            for h2 in range(2):
                                tkc = tk2 * 2 + h2
                                s_ps = psSp.tile([128, 512], FP, tag="s",
                                                 name="sps")
                                nc.tensor.matmul(
                                    s_ps,
                                    kt[pb:pb + 64, fc,
                                       tkc * 128:(tkc + 1) * 128],
                                    qt[pb:pb + 64, fc,
                                       qh * 512:(qh + 1) * 512],
                                    start=True, stop=True)
                                nc.scalar.activation(
                                    out=praw[:, h2 * 512:(h2 + 1) * 512],
                                    in_=s_ps, func=AF.Exp, scale=0.125)
                        ptt = ptp.tile([128, 1024], BF, tag="pt", name="ptt")
                        nc.gpsimd.tensor_tensor(
                            out=ptt.rearrange("p (a b) -> p a b", b=512),
                            in0=praw.rearrange("p (a b) -> p a b", b=512),
                            in1=amt[:, tk2 * 2:tk2 * 2 + 2,
                                    qh * 512:(qh + 1) * 512],
                            op=AL.mult)
                        for h2 in range(2):
                            tkc = tk2 * 2 + h2
                            for t4 in range(4):
                                nc.tensor.matmul(
                                    po[:, t4 * (HD + 1):(t4 + 1) * (HD + 1)],
                                    ptt[:, h2 * 512 + t4 * 128:
                                        h2 * 512 + (t4 + 1) * 128],
                                    vt[tkc][:, hh, :],
                                    start=False, stop=(tkc == 7),
                                    skip_group_check=True)
                # combined drain
                rd = smallp.tile([128, 8], FP, tag="rd", name="rd")
                onorm = smallp.tile([128, 512], BF, tag="on", name="onorm")
                for sub in range(2):
                    poap = pos[sub][:]
                    nc.vector.reciprocal(
                        out=rd[:, 4 * sub:4 * sub + 4],
                        in_=sub_ap(poap, HD, [[HD + 1, 4]]))
                    for t4 in range(4):
                        nc.vector.tensor_scalar(
                            out=onorm[:, t4 * 128 + sub * HD:
                                      t4 * 128 + (sub + 1) * HD],
                            in0=pos[sub][:, t4 * (HD + 1):t4 * (HD + 1) + HD],
                            scalar1=rd[:, 4 * sub + t4:4 * sub + t4 + 1],
                            scalar2=None, op0=AL.mult)
                nc.sync.dma_start_transpose(
                    ot[:, fc, qh * 512:(qh + 1) * 512].rearrange(
                        "p (a b) -> p a b", b=128),
                    onorm)

            # NOTE: interleaving conv chunks with attention pairs on the PE
            # stream crashes real hardware (sim-clean); keep them sequential.
            with tc.tile_pool(name="psvB", bufs=2, space="PSUM") as psvB:
                for oc4 in range(4, 8):
                    conv_chunk(xtq, wq_d, oc4, drain_q, psvB)
            with tc.tile_pool(name="psS", bufs=2, space="PSUM") as psS:
                for fc in range(8):
                    attn_pair(0, fc, psS, True)

            # qh=1 heads interleaved with first half of the output linear
            with tc.tile_pool(name="linp", bufs=1) as linp, \
                    tc.tile_pool(name="stg2", bufs=2) as stg2, \
                    tc.tile_pool(name="psL", bufs=1, space="PSUM") as psL:
                wos = [linp.tile([128, C], BF, name=f"wos{i}") for i in range(8)]
                for fcc in range(8):
                    nc.sync.dma_start(
                        out=wos[fcc], in_=wo_d[fcc * 128:(fcc + 1) * 128, :])
                bob = linp.tile([128, C], FP)
                nc.gpsimd.dma_start(out=bob, in_=bcast(bo_d))

                def linear_tcc(tcc, pool):
                    pl = pool.tile([128, C], FP, tag="l", name="pl")
                    for fcc in range(8):
                        lhsT = ot[:, fcc, tcc * 128:(tcc + 1) * 128]
                        for jh in range(2):
                            nc.tensor.matmul(
                                pl[:, jh * 512:(jh + 1) * 512],
                                lhsT,
                                wos[fcc][:, jh * 512:(jh + 1) * 512],
                                start=(fcc == 0), stop=(fcc == 7))
                    so = stg2.tile([128, C], FP, tag="so", name="so")
                    nc.vector.tensor_add(so, pl, bob)
                    nc.scalar.dma_start(
                        out=out_d[tcc * 128:(tcc + 1) * 128, :], in_=so)

                with tc.tile_pool(name="psS2", bufs=2,
                                  space="PSUM") as psS2:
                    for fc in range(8):
                        attn_pair(1, fc, psS2, True)
                        if fc % 2 == 1:
                            linear_tcc(fc // 2, psL)
                with tc.tile_pool(name="psL2", bufs=2, space="PSUM") as psL2:
                    for tcc in range(4, 8):
                        linear_tcc(tcc, psL2)


def _prep_w(W):
    """[O, I, 3, 3] fp32 -> [32, 128, 4608] fp8 tiles.

    Tile t = oc4*4+icc2, per-partition content [icc(2), hilo(2), s(9), oc(128)].
    """
    import ml_dtypes
    E4 = ml_dtypes.float8_e4m3
    Ws = np.asarray(W, np.float32) * SW
    W0 = Ws.astype(E4)
    W1 = (Ws - W0.astype(np.float32)).astype(E4)
    # [hilo, O, I, 3*3] -> [hilo, I, s, O]
    pair = np.stack([W0, W1]).reshape(2, 1024, 1024, 9).transpose(0, 2, 3, 1)
    # I = icc2*256 + icc*128 + p ; O = oc4*128 + oc
    pair = pair.reshape(2, 4, 2, 128, 9, 8, 128)     # [h, icc2, icc, p, s, oc4, oc]
    out = pair.transpose(5, 1, 3, 2, 0, 4, 6)        # [oc4, icc2, p, icc, h, s, oc]
    return np.ascontiguousarray(out.reshape(32, 128, 4608))


def _prep_x(x):
    """[C, 32, 32] fp32 -> [128, 3, 8, PA] fp8 (slots x0, x1, x0)."""
    import ml_dtypes
    E4 = ml_dtypes.float8_e4m3
    xp = np.zeros((C, PADH, PADW), np.float32)
    xp[:, 1:33, 1:33] = np.asarray(x, np.float32)
    xs = (xp * SX).reshape(C, PA)
    x0 = xs.astype(E4)
    x1 = (xs - x0.astype(np.float32)).astype(E4)
    tri = np.stack([x0, x1, x0], axis=1)             # [C, 3, PA]
    tri = tri.reshape(8, 128, 3, PA).transpose(1, 0, 2, 3)  # [p, icc, slot, PA]
    return np.ascontiguousarray(tri)


def get_program():
    if "nc" not in _CACHE:
        _CACHE["nc"] = _build_program()
    return _CACHE["nc"]


def make_in_maps(q, k, v, Wq, bq, Wk, bk, Wv, bv, Wo, bo, mask):
    import ml_dtypes
    BF16 = ml_dtypes.bfloat16
    wq = _prep_w(Wq)
    wk = _prep_w(Wk)
    wv = _prep_w(Wv)
    wo = np.ascontiguousarray(np.asarray(Wo, np.float32).T.astype(BF16))
    bq, bk, bv, bo = (np.ascontiguousarray(np.asarray(b), dtype=np.float32)
                      for b in (bq, bk, bv, bo))
    in_maps = []
    for b in range(B):
        mt = np.asarray(mask[b]).T.astype(np.float32).astype(BF16)
        in_maps.append({
            "xq": _prep_x(np.asarray(q[b]).reshape(C, 32, 32)),
            "xk": _prep_x(np.asarray(k[b]).reshape(C, 32, 32)),
            "xv": _prep_x(np.asarray(v[b]).reshape(C, 32, 32)),
            "wq": wq, "wk": wk, "wv": wv, "wo": wo,
            "bq": bq, "bk": bk, "bv": bv, "bo": bo,
            "mt": np.ascontiguousarray(mt),
        })
    return in_maps


def run(inputs, trace=False, **kw):
    from concourse.bass_utils import run_bass_kernel_spmd

    nc = get_program()
    in_maps = make_in_maps(**inputs)
    res = run_bass_kernel_spmd(nc, in_maps, list(range(B)), trace=trace, **kw)
    out = np.stack([res.results[i]["out"] for i in range(B)], axis=0)
    return out, res


def kernel(**inputs) -> np.ndarray:
    out, _ = run(inputs, trace=False)
    return out
